# revision 33
# baseline (speedup 1.0000x reference)
"""AnchorDML Trainium2 kernel: 8-core SPMD, data-parallel over x rows with
sharded anchor encoding + AllGather of encoded anchors.

Problem (hardcoded):
    N, M, D, C = 8192, 4096, 512, 100
    xe = mish(mish(x @ W1 + b1) @ W2 + b2)          [N, D]
    se = mish(mish(samples @ W1 + b1) @ W2 + b2)    [M, D]
    dist = sqrt(max(|xe|^2 + |se|^2 - 2 xe@se.T, 0))  [N, M]
    out = log_softmax(tanh(dist @ Wp + bp), axis=1)   [N, C]

Sharding: core g handles x rows [1024g, 1024(g+1)) and encodes anchors
[512g, 512(g+1)); encoded (scaled) anchors + |se|^2 are AllGathered.

v2 design (vs the 171us exp/ln/tanh-mish version):
 - mish(v) ~= v * sigmoid(A*v + B) (A=1.3058, B=0.5054, minimax over the
   observed pre-activation distribution): ONE table-resident ACT pass per
   layer instead of three (Exp, Ln, Tanh), no per-phase table flapping.
   End-to-end relL2 vs the exact reference measured at ~2e-3 in numpy
   (gate is 2e-2).
 - encoder matmuls run fp8(e4m3) DoubleRow: W1,W2 are host-scaled by 16
   (power of 2, exactly undone in the ACT scale and the DVE mult scalar),
   hidden h is carried as 8*mish in fp8. Per-layer: 2 DR matmuls per
   128-col output chunk + 1 f32r bias-row matmul (exact b handling),
   then sigmoid on a [128,2,512] PSUM pair + one fused DVE
   (c*v)*sigma multiply that also converts to fp8.
 - d2 tiles are PSUM pairs [128,2,512] (both 512-row chunks of an anchor
   tile): one DVE +|xe|^2 add and one ACT sqrt (with |se|^2 as the
   per-partition bias) per pair, halving per-instruction overheads.
 - 4 ACT table loads total: sigmoid_and_others (encoders; Square for
   |se|^2 is co-resident), sqrt_and_others (distance), back to sigmoid
   (epilogue tanh), natural_log_exp (softmax exp+ln).
 - one AllGather carries fp8 anchors + packed f32 |se|^2 rows (as in v1),
   with a tiny warmup collective to wake the ncfw firmware early.
"""
import numpy as np
import ml_dtypes
from concourse import bass, bacc, tile, mybir, bass_utils, masks

N, M, D, C = 8192, 4096, 512, 100
NCORES = 8
RPC = N // NCORES      # 1024 x-rows per core
MPC = M // NCORES      # 512 anchors encoded per core
KD = D // 128          # 4 contraction chunks of 128
NMT = M // 128         # 32 anchor tiles in the distance matmul
TPG = MPC // 128       # 4 anchor tiles per gathered rank
AGW = MPC // 2         # ag row width in bf16 units (fp8 pairs)
AGR = D + 4            # 512 fp8 anchor rows + 4 rows of packed fp32 s2
LAG = 3                # zT matmul trails the d2 pairs by LAG pairs

A_SIG = 1.3058         # mish(v) ~= v * sigmoid(A_SIG*v + B_SIG)
B_SIG = 0.5054
SW = 16.0              # host scale on W1/W2 (exact power of 2)
SH = 8.0               # scale on the fp8 hidden activations

F32 = mybir.dt.float32
F32R = mybir.dt.float32r
BF16 = mybir.dt.bfloat16
FP8 = mybir.dt.float8e4
AF = mybir.ActivationFunctionType
ALU = mybir.AluOpType
DR = mybir.MatmulPerfMode.DoubleRow


def _patched_tables(arch):
    """Pin each used table-based ACT function to exactly one table set so
    the kernel pays 4 table loads total: Sigmoid+Tanh (+Square, Copy)
    live in sigmoid_and_others, Sqrt in sqrt_and_others, Exp+Ln in
    natural_log_exp_and_others. (Dict order is preserved --
    act_func_set_id is positional.)"""
    from concourse.hw_specs import get_activation_tables as orig
    out = {}
    for name, s in orig(arch).items():
        s = set(s)
        if name != "sigmoid_and_others":
            s.discard(AF.Sigmoid)
            s.discard(AF.Tanh)
        if name != "sqrt_and_others":
            s.discard(AF.Sqrt)
        if name != "natural_log_exp_and_others":
            s.discard(AF.Exp)
            s.discard(AF.Ln)
        out[name] = s
    return out


def build_kernel(with_bias=False):
    bacc.get_activation_tables = _patched_tables
    nc = bacc.Bacc("TRN2", target_bir_lowering=False, debug=False,
                   num_devices=NCORES)

    eT = nc.dram_tensor("eT", [D, MPC + RPC], FP8, kind="ExternalInput")
    W1 = nc.dram_tensor("W1", [D, D], FP8, kind="ExternalInput")
    W2 = nc.dram_tensor("W2", [D, D], FP8, kind="ExternalInput")
    b1r = nc.dram_tensor("b1r", [2, D], F32, kind="ExternalInput")
    b2r = nc.dram_tensor("b2r", [1, D], F32, kind="ExternalInput")
    Wp = nc.dram_tensor("Wp", [M, C], F32, kind="ExternalInput")
    bp = nc.dram_tensor("bp", [1, C], F32, kind="ExternalInput")
    out = nc.dram_tensor("out", [RPC, C], F32, kind="ExternalOutput")

    with tile.TileContext(nc) as tc:
        _body(tc, eT, W1, W2, b1r, b2r, Wp, bp, out, with_bias)

    nc.compile()
    return nc


def _body(tc, eT, W1, W2, b1r, b2r, Wp, bp, out, with_bias):
    nc = tc.nc
    with (
        tc.tile_pool(name="const", bufs=1) as const,
        tc.tile_pool(name="wpool", bufs=1) as wpool,
        tc.tile_pool(name="spool", bufs=1) as spool,
        tc.tile_pool(name="xpool", bufs=1) as xpool,
        tc.tile_pool(name="gpool", bufs=1) as gpool,
        tc.tile_pool(name="mpool", bufs=3) as mpool,
        tc.tile_pool(name="dpool", bufs=1) as dpool,
        tc.tile_pool(name="zpool", bufs=2) as zpool,
        tc.tile_pool(name="ps", bufs=1, space="PSUM") as ps,
        tc.tile_pool(name="psz", bufs=1, space="PSUM") as psz,
        tc.tile_pool(name="dram", bufs=1, space="DRAM") as dram,
    ):
        # ---- warmup collective staging heads the pool queue (128B, in
        # flight by ~10us even under input-load traffic) ----
        warm_sb = const.tile([1, 64], BF16)
        nc.gpsimd.memset(warm_sb[:], 1.0)
        warm_in = dram.tile([1, 64], BF16)
        warm_out = dram.tile([NCORES, 64], BF16, addr_space="Shared")
        nc.gpsimd.dma_start(warm_in[:], warm_sb[:])
        nc.gpsimd.collective_compute(
            "AllGather", ALU.bypass,
            replica_groups=[list(range(NCORES))],
            ins=[warm_in.opt()], outs=[warm_out.opt()])

        # ---- input loads: eTs + W1 gate the first anchor matmul and
        # head the queues; W2/eTx/Wp issue is gated below. ----
        eTs_sb = spool.tile([128, KD, MPC], FP8)
        nc.sync.dma_start(eTs_sb[:, :, :],
                          eT[:, :MPC].rearrange("(k p) m -> p k m", p=128))
        W1_sb = wpool.tile([128, KD, D], FP8)
        nc.sync.dma_start(W1_sb[:, :, :],
                          W1[:].rearrange("(k p) d -> p k d", p=128))
        b1r_sb = wpool.tile([1, D], F32R)
        nc.sync.dma_start(b1r_sb[:], b1r[0:1, :].bitcast(F32R))
        onesr_sb = wpool.tile([1, D], F32R)      # ones row (from b1r row 1)
        nc.sync.dma_start(onesr_sb[:], b1r[1:2, :].bitcast(F32R))
        b2r_sb = wpool.tile([1, D], F32R)
        if with_bias:
            nc.gpsimd.dma_start(b2r_sb[:], b2r[:].bitcast(F32R))

        W2_sb = wpool.tile([128, KD, D], FP8)
        w2_dma = nc.sync.dma_start(W2_sb[:, :, :],
                                   W2[:].rearrange("(k p) d -> p k d", p=128))
        eTx_sb = xpool.tile([128, KD, RPC], FP8)
        etx_dmas = []
        for rc in range(2):
            etx_dmas.append(nc.scalar.dma_start(
                eTx_sb[:, :, 512 * rc:512 * (rc + 1)],
                eT[:, MPC + 512 * rc:MPC + 512 * (rc + 1)].rearrange(
                    "(k p) m -> p k m", p=128)))

        # ---- constants ----
        ident = const.tile([C, C], F32)
        masks.make_identity(nc, ident[:])
        onesq_col = const.tile([128, 1], BF16)   # 0.25: undoes (-2)^2 in s2
        nc.gpsimd.memset(onesq_col[:], 0.25)
        ones_col = const.tile([128, 1], BF16)    # 1.0: x2 row sums
        nc.gpsimd.memset(ones_col[:], 1.0)
        bsig_col = const.tile([128, 1], F32)     # sigmoid bias B_SIG
        nc.gpsimd.memset(bsig_col[:], B_SIG)
        # preload the sigmoid table while input DMAs are in flight
        tdum = const.tile([128, 1], BF16)
        nc.scalar.activation(tdum[:], bsig_col[:], AF.Sigmoid)
        bp_sb = wpool.tile([1, C], F32R)
        nc.gpsimd.dma_start(bp_sb[:], bp[:].bitcast(F32R))

        Wp_sb = wpool.tile([128, NMT, C], F32R)
        wp_dma = nc.gpsimd.dma_start(
            Wp_sb[:, :, :],
            Wp[:].bitcast(F32R).rearrange("(t p) c -> p t c", p=128))

        ones_r = onesr_sb[0:1, :512]

        def enc_phase(dst, Wsb, brow, src, src_off, width, cmul, sscale,
                      on_pair=None):
            """dst[:, :, src_off:src_off+width] = fp8(cmul * P * sigmoid(
            sscale * P + B_SIG)) where P = (scaled W)^T src + scaled-b,
            accumulated over KD fp8-DR chunks + one f32r bias row.
            Emits per [128,2,512] PSUM pair: 4 DR + 2 bias matmuls, one
            sigmoid, one fused DVE multiply (also the fp8 convert)."""
            first_mm, first_sig, first_stt = [], [], []
            cw = min(width, 512)
            for w in range(width // cw):
                ssl = slice(src_off + cw * w, src_off + cw * (w + 1))
                for p in range(2):
                    P = ps.tile([128, 2, 512], F32, tag="mm", bufs=3)
                    sig = mpool.tile([128, 2, 512], BF16, tag="sig")
                    for f2 in range(2):
                        f = 2 * p + f2
                        for q in range(2):
                            mm_i = nc.tensor.matmul(
                                P[:, f2, :cw],
                                Wsb[:, 2 * q:2 * q + 2, 128 * f:128 * (f + 1)],
                                src[:, 2 * q:2 * q + 2, ssl],
                                start=(q == 0),
                                stop=(q == 1 and not with_bias),
                                perf_mode=DR)
                            if not first_mm:
                                first_mm.append(mm_i)
                        if with_bias:
                            nc.tensor.matmul(
                                P[:, f2, :cw],
                                brow[0:1, 128 * f:128 * (f + 1)],
                                ones_r[:, :cw], start=False, stop=True)
                        s_i = nc.scalar.activation(sig[:, f2, :cw],
                                                   P[:, f2, :cw], AF.Sigmoid,
                                                   bias=bsig_col[:],
                                                   scale=sscale)
                        if not first_sig:
                            first_sig.append(s_i)
                        t_i = nc.vector.scalar_tensor_tensor(
                            dst[:, 2 * p + f2, ssl], P[:, f2, :cw], cmul,
                            sig[:, f2, :cw], op0=ALU.mult, op1=ALU.mult)
                        if not first_stt:
                            first_stt.append(t_i)
                    if on_pair is not None:
                        on_pair(p)
            return {"first_mm": first_mm[0], "first_sig": first_sig[0],
                    "first_stt": first_stt[0], "last_sig": s_i,
                    "last_stt": t_i}

        # ---- anchor encode first, both layers, so the AllGather issues
        # as early as possible. h = 8*mish (fp8), seA = -2*mish (fp8). ----
        h_se = spool.tile([128, KD, MPC], FP8)
        se1_h = enc_phase(h_se, W1_sb, b1r_sb, eTs_sb, 0, MPC,
                          SH / SW, A_SIG / SW)
        seA_sb = spool.tile([128, KD, MPC], FP8)
        sqse_sb = spool.tile([128, KD, MPC], BF16)
        # s2 sums live in a slice of the (idle until the main loop) zT
        # PSUM bank: a tag-"mm" tile here would pin a ring slot across the
        # whole anchor phase and serialize the x encode behind it
        zt_ps = [psz.tile([C, 512], F32, name=f"ztps{rc}") for rc in range(2)]
        s2p = zt_ps[0]
        sq_state = {}

        ag_in = dram.tile([AGR, AGW], BF16)
        ag_out = dram.tile([NCORES * AGR, AGW], BF16, addr_space="Shared")

        def se2_pair(p):
            # stage this pair's fp8 chunks for the collective immediately,
            # then Square + partial s2 sums (the s2 rows stage last)
            nc.scalar.dma_start(
                ag_in[256 * p:256 * (p + 1), :].rearrange(
                    "(k p2) m -> p2 k m", p2=128),
                seA_sb[:, 2 * p:2 * p + 2, :].bitcast(BF16))
            sq_state["sq"] = nc.scalar.activation(
                sqse_sb[:, 2 * p:2 * p + 2, :], seA_sb[:, 2 * p:2 * p + 2, :],
                AF.Square)
            for k in (2 * p, 2 * p + 1):
                sq_state["mm"] = nc.tensor.matmul(
                    s2p[0:1, :], onesq_col[:], sqse_sb[:, k, :],
                    start=(k == 0), stop=(k == KD - 1))

        se2_h = enc_phase(seA_sb, W2_sb, b2r_sb, h_se, 0, MPC,
                          -2.0 / (SW * SH), A_SIG / (SW * SH),
                          on_pair=se2_pair)
        sq_se_i = sq_state["sq"]
        s2_mm_i = sq_state["mm"]
        s2row_sb = spool.tile([1, MPC], F32)
        s2row_i = nc.vector.tensor_copy(s2row_sb[:], s2p[0:1, :])
        nc.scalar.dma_start(
            ag_in[D:AGR, :].rearrange("(o a) m -> o (a m)", o=1),
            s2row_sb[:].bitcast(BF16))
        nc.gpsimd.collective_compute(
            "AllGather", ALU.bypass,
            replica_groups=[list(range(NCORES))],
            ins=[ag_in.opt()], outs=[ag_out.opt()])
        tile.add_dep_helper(w2_dma.ins, se1_h["first_mm"].ins, sync=True,
                            reason="W2 issue after anchor L1 starts")
        for e in etx_dmas:
            tile.add_dep_helper(e.ins, se1_h["first_mm"].ins, sync=True,
                                reason="eTx issue after anchor L1 starts")
        tile.add_dep_helper(wp_dma.ins, se2_h["first_mm"].ins, sync=True,
                            reason="Wp issue after anchor L2 starts")

        # ---- x encode overlaps the AllGather; per 512-row half the
        # |xe|^2 row follows immediately so loop inputs are ready early ----
        h_xe = xpool.tile([128, KD, RPC], FP8)
        xe_sb = xpool.tile([128, KD, RPC], FP8)
        x2row_sb = xpool.tile([1, RPC], F32R)
        x2b_sb = xpool.tile([128, 2, 512], F32)

        def x2_phase(rc):
            rsl = slice(512 * rc, 512 * (rc + 1))
            sqxe = xpool.tile([128, KD, 512], BF16, tag=f"sqxe{rc}")
            nc.vector.tensor_tensor(sqxe[:, :, :], xe_sb[:, :, rsl],
                                    xe_sb[:, :, rsl], op=ALU.mult)
            xp = ps.tile([128, 2, 512], F32, tag="mm", bufs=3)
            for k in range(KD):
                nc.tensor.matmul(xp[0:1, 0, :], ones_col[:], sqxe[:, k, :],
                                 start=(k == 0), stop=(k == KD - 1))
            nc.vector.tensor_copy(x2row_sb[0:1, rsl], xp[0:1, 0, :])
            xb = ps.tile([128, 2, 512], F32, tag="mm", bufs=3)
            nc.tensor.matmul(xb[:, 0, :], onesr_sb[0:1, :128],
                             x2row_sb[0:1, rsl],
                             start=True, stop=True)
            nc.vector.tensor_copy(x2b_sb[:, rc, :], xb[:, 0, :])

        xe1_h = enc_phase(h_xe, W1_sb, b1r_sb, eTx_sb, 0, RPC,
                          SH / SW, A_SIG / SW)
        xe2_h = enc_phase(xe_sb, W2_sb, b2r_sb, h_xe, 0, 512,
                          1.0 / (SW * SH), A_SIG / (SW * SH))
        x2_phase(0)
        xe2b_h = enc_phase(xe_sb, W2_sb, b2r_sb, h_xe, 512, 512,
                           1.0 / (SW * SH), A_SIG / (SW * SH))
        x2_phase(1)
        tile.add_dep_helper(xe1_h["first_sig"].ins, sq_se_i.ins, sync=False,
                            reason="anchor ACT chain before x-side ACT")
        tile.add_dep_helper(xe1_h["first_stt"].ins, s2row_i.ins, sync=False,
                            reason="anchor DVE chain before x-side DVE")
        tile.add_dep_helper(xe1_h["first_mm"].ins, s2_mm_i.ins, sync=False,
                            reason="anchor PE chain + s2 sums before x-side PE")

        # ---- load gathered anchors: per-rank descriptors rotate across
        # three DMA queues; rank 0's s2 heads its queue. ----
        s2c_sb = gpool.tile([128, NCORES, TPG], F32)
        seAg_sb = gpool.tile([128, NCORES, KD, MPC], FP8)
        qs = [nc.sync, nc.gpsimd, nc.scalar]
        for g in range(NCORES):
            q = qs[g % 3]
            descs = [
                (s2c_sb[:, g, :],
                 ag_out[AGR * g + D:AGR * (g + 1), :].bitcast(F32)
                 .rearrange("a p -> p a")),
                (seAg_sb[:, g, :, :].bitcast(BF16),
                 ag_out[AGR * g:AGR * g + D, :].rearrange(
                     "(k p) m -> p k m", p=128)),
            ]
            for dst, src_ in (descs if g < 2 else descs[::-1]):
                q.dma_start(dst, src_)

        # ---- main fused loop over anchor-tile PSUM pairs: both 512-row
        # chunks of tile t share one DVE add and one ACT sqrt (s2 rides
        # as the per-partition bias). zT trails by LAG pairs. ----
        for rc in range(2):
            nc.tensor.matmul(zt_ps[rc][:], bp_sb[:], ones_r,
                             start=True, stop=False, skip_group_check=True)
        dist_tiles = {}
        first_sqrt = []
        order = [(g, tl) for g in range(NCORES) for tl in range(TPG)]
        for ti, (g, tl) in enumerate(order):
            sA = seAg_sb[:, g, :, :]
            bias = s2c_sb[:, g, tl:tl + 1]
            P = ps.tile([128, 2, 512], F32, tag="mm", bufs=3)
            for q in range(2):
                for rc in range(2):
                    nc.tensor.matmul(
                        P[:, rc, :],
                        sA[:, 2 * q:2 * q + 2, 128 * tl:128 * (tl + 1)],
                        xe_sb[:, 2 * q:2 * q + 2, 512 * rc:512 * (rc + 1)],
                        start=(q == 0), stop=(q == 1), perf_mode=DR)
            nc.vector.tensor_tensor(P[:, :, :], P[:, :, :], x2b_sb[:, :, :],
                                    op=ALU.add)
            dp = dpool.tile([128, 2, 512], F32R, tag="dist", bufs=5)
            sq_i = nc.scalar.activation(dp[:], P[:], AF.Sqrt, bias=bias)
            if not first_sqrt:
                first_sqrt.append(sq_i)
                tile.add_dep_helper(
                    sq_i.ins, xe2b_h["last_sig"].ins, sync=False,
                    reason="sqrt table load after the last encoder ACT")
            dist_tiles[ti] = (ti, dp)
            if ti >= LAG:
                tz, dpl = dist_tiles.pop(ti - LAG)
                for rc in range(2):
                    nc.tensor.matmul(zt_ps[rc][:], Wp_sb[:, tz, :],
                                     dpl[:, rc, :],
                                     start=False, stop=False,
                                     skip_group_check=True)
        NPAIRS = NMT
        for ti in range(NPAIRS - LAG, NPAIRS):
            tz, dpl = dist_tiles.pop(ti)
            for rc in range(2):
                nc.tensor.matmul(zt_ps[rc][:], Wp_sb[:, tz, :],
                                 dpl[:, rc, :],
                                 start=False, stop=(ti == NPAIRS - 1),
                                 skip_group_check=True)

        # ---- epilogue: tanh on the bias'd zT ([C, 512] layout), PE
        # transposes carry tanh'd values, then one batched log-softmax
        # (tanh output is in [-1,1]: no max-subtraction needed) ----
        NT = 8
        zth_sb = zpool.tile([128, NT, C], BF16, bufs=1)
        for rc in range(2):
            zt_sb = zpool.tile([C, 512], F32, bufs=2, tag="ztsb")
            nc.scalar.activation(zt_sb[:], zt_ps[rc][:], AF.Tanh)
            for j in range(4):
                ztr = ps.tile([128, 2, 512], F32, tag="mm", bufs=3)
                nc.tensor.matmul(ztr[:, 0, :C],
                                 zt_sb[:, 128 * j:128 * (j + 1)],
                                 ident[:], is_transpose=True)
                nc.vector.tensor_copy(zth_sb[:, 4 * rc + j, :],
                                      ztr[:, 0, :C])
        e_sb = zpool.tile([128, NT, C], BF16, bufs=1)
        nc.scalar.activation(e_sb[:, :, :], zth_sb[:, :, :], AF.Exp)
        ssum = zpool.tile([128, NT], F32, bufs=1)
        nc.vector.tensor_reduce(ssum[:], e_sb[:, :, :],
                                axis=mybir.AxisListType.X, op=ALU.add)
        lns = zpool.tile([128, NT], F32, bufs=1)
        nc.scalar.activation(lns[:], ssum[:], AF.Ln)
        for rc in range(2):
            o_sb = zpool.tile([128, 4, C], F32, bufs=2, tag="osb")
            for j in range(4):
                jj = 4 * rc + j
                nc.vector.tensor_scalar(o_sb[:, j, :], zth_sb[:, jj, :],
                                        lns[:, jj:jj + 1], None,
                                        op0=ALU.subtract)
            nc.sync.dma_start(
                out[512 * rc:512 * (rc + 1), :].rearrange(
                    "(j p) c -> p j c", p=128),
                o_sb[:, :, :])


_NC_CACHE = {}


def _get_nc(with_bias=False):
    if with_bias not in _NC_CACHE:
        _NC_CACHE[with_bias] = build_kernel(with_bias)
    return _NC_CACHE[with_bias]


def make_in_maps(x, samples, W1, b1, W2, b2, Wp, bp):
    f8 = ml_dtypes.float8_e4m3fn
    x = np.asarray(x, dtype=np.float32)
    samples = np.asarray(samples, dtype=np.float32)
    W1q = np.ascontiguousarray(
        (np.asarray(W1, dtype=np.float32) * SW).astype(f8))
    W2q = np.ascontiguousarray(
        (np.asarray(W2, dtype=np.float32) * SW).astype(f8))
    b1c = np.ascontiguousarray(np.stack([
        np.asarray(b1, dtype=np.float32).reshape(D) * SW,
        np.ones(D, dtype=np.float32)]))
    b2c = np.ascontiguousarray(
        (np.asarray(b2, dtype=np.float32) * SW * SH).reshape(1, D))
    Wpf = np.asarray(Wp, dtype=np.float32)
    bpc = np.ascontiguousarray(np.asarray(bp, dtype=np.float32).reshape(1, C))
    in_maps = []
    for g in range(NCORES):
        sT_g = samples[MPC * g:MPC * (g + 1), :].T
        xT_g = x[RPC * g:RPC * (g + 1), :].T
        eT_g = np.concatenate([sT_g, xT_g], axis=1).astype(f8)
        in_maps.append({
            "eT": np.ascontiguousarray(eT_g),
            "W1": W1q, "W2": W2q, "b1r": b1c, "b2r": b2c,
            "Wp": np.ascontiguousarray(Wpf), "bp": bpc,
        })
    return in_maps


def run(in_maps, trace=False):
    with_bias = bool(np.any(in_maps[0]["b1r"]) or np.any(in_maps[0]["b2r"]))
    nc = _get_nc(with_bias)
    res = bass_utils.run_bass_kernel_spmd(nc, in_maps,
                                          core_ids=list(range(NCORES)),
                                          trace=trace)
    outp = np.concatenate([res.results[g]["out"] for g in range(NCORES)],
                          axis=0).astype(np.float32)
    return outp, res


def kernel(x, samples, W1, b1, W2, b2, Wp, bp):
    in_maps = make_in_maps(x, samples, W1, b1, W2, b2, Wp, bp)
    outp, _ = run(in_maps, trace=False)
    return outp
